# revision 13
# baseline (speedup 1.0000x reference)
"""Trainium2 Bass kernel for GQA multi-head attention (B=4, S=2048, HID=1280,
NH=16, NKV=4, HD=80) sharded over 8 NeuronCores as (batch x kv-head-group).

Per core (b, hg): 8 q heads / 2 kv heads of batch b.
  A1: Q/K projection, d-major (fp32r matmuls) -> Q_T[d, h, t], K_T[d, kv, t] bf16
  A2: V projection, token-major (bf16 matmuls) -> V'[t, kc, kv*81] + ones column
  B:  scores (bf16) -> exp (ACT) -> causal 0/1 mask mul (gpsimd)
      -> PV matmul with ones-row denominator -> normalize (recip+bcast+mul)
  D:  o_proj row-parallel partial (bf16); host sums the two head-group partials.
"""

import functools
import math

import numpy as np
import ml_dtypes

import concourse.bass as bass
import concourse.mybir as mybir
import concourse.tile as tile
from concourse import bacc

B, S, HID = 4, 2048, 1280
NH, NKV, HD = 16, 4, 80
G = NH // NKV  # 4
Q_SIZE, KV_SIZE = NH * HD, NKV * HD
NCORE = 8
HL = 8          # local q heads per core
KVL = 2         # local kv heads per core
LQ = HL * HD    # 640 local q cols
LKV = KVL * HD  # 160 local k (and v) cols
NQKV = LQ + 2 * LKV  # 960 local qkv cols

F32 = mybir.dt.float32
F32R = mybir.dt.float32r
BF16 = mybir.dt.bfloat16

TB = 512        # stage-A token block
QB = 512        # stage-B q block
KC = 128        # k chunk (partitions)


def _build(s, causal, bias, rep=1, loop_n=0):
    """Build + compile the per-core Bass program. Same program on all cores.

    v2 schedule: stage-B (attention) is ACT-bound (exp); stage-A/D matmul
    "filler units" are interleaved between attention heads so the PE never
    idles waiting on the exp pipeline.  DMA queues are split: loads +
    latency-critical SBUF repacks on the sync (SP) HWDGE queue, stores and
    o_pk packs on the gpsimd SWDGE queue.
    """
    nqb = s // QB
    ntb = s // TB
    nkc_hid = HID // KC  # 10
    n_tc = s // 128
    tb_per_qb = QB // TB   # 1
    kc_per_qb = QB // KC   # 4
    assert TB == QB

    nc = bacc.Bacc(None)
    xt = nc.declare_dram_parameter("xt", [HID + (1 if bias else 0), s], BF16,
                                   isOutput=False)
    # wtm: host-prepacked m-major qk weight chunks [m, p, c, j] so each
    # m-chunk loads as one full-bandwidth DMA (contiguous per partition)
    wtm = nc.declare_dram_parameter("wtm", [7, 128, HID // KC, 128], BF16,
                                    isOutput=False)
    wtvp = nc.declare_dram_parameter("wtvp", [128, HID // KC, LKV], BF16,
                                     isOutput=False)
    if bias:
        wtb = nc.declare_dram_parameter("wtb", [1, NQKV], BF16,
                                        isOutput=False)
    owt = nc.declare_dram_parameter("owt", [LQ, HID], BF16, isOutput=False)
    if causal:
        m01 = nc.declare_dram_parameter("m01", [QB // KC, KC, QB], BF16,
                                        isOutput=False)
    else:
        m01 = nc.declare_dram_parameter("m01", [s // KC, KC, s], BF16,
                                        isOutput=False)
    out = nc.declare_dram_parameter("out", [s, HID], F32, isOutput=True)

    nkc_a = nkc_hid + (1 if bias else 0)
    xt_r = xt[0:HID, :].rearrange("(c p) t -> p c t", p=128)
    VW = 97  # 80 v cols + 16 zero pad + ones col at 96
    JBS = [(0, 512), (512, 512), (1024, 256)]

    with tile.TileContext(nc) as tc:
        with (
            tc.tile_pool(name="persist", bufs=1) as persist,
            tc.tile_pool(name="wtp", bufs=1) as wtp,
            tc.tile_pool(name="xtp", bufs=2) as xtp,
            tc.tile_pool(name="bwork", bufs=2) as bwork,
            tc.tile_pool(name="dstage", bufs=2) as dstage,
            tc.tile_pool(name="psA", bufs=2, space="PSUM") as psA,
            tc.tile_pool(name="psSC", bufs=2, space="PSUM") as psSC,
            tc.tile_pool(name="psPV", bufs=2, space="PSUM") as psPV,
        ):
            # ---- persistent SBUF, split per qb-block for fine-grained deps
            q_l = [persist.tile([80, HL, QB], BF16, name=f"q{j}")
                   for j in range(nqb)]
            k_l = [persist.tile([80, KVL, QB], BF16, name=f"k{j}")
                   for j in range(nqb)]
            v_l = [persist.tile([128, kc_per_qb, 2 * VW], BF16, name=f"v{j}")
                   for j in range(nqb)]
            o_pk = [persist.tile([128, LQ // 128, QB], BF16, name=f"opk{j}")
                    for j in range(nqb)]
            qkpk_l = [None] * nqb
            for j in range(nqb):
                nc.vector.memset(v_l[j][:], 0.0)
                nc.vector.memset(v_l[j][:, :, 96:97], 1.0)
                nc.vector.memset(v_l[j][:, :, VW + 96:VW + 97], 1.0)

            # ---- stage A weights.  m-chunk 0 goes first on the sync queue
            # (the first A1 matmul needs it + the first xtb block); the rest
            # stream on the scalar-engine HWDGE queue in parallel.  mask/owt
            # (needed much later) are emitted last so they queue behind the
            # startup-critical loads on the shared DMA engines.
            wt_sb = wtp.tile([128, nkc_hid, LQ + LKV], BF16)
            nc.sync.dma_start(wt_sb[:, :, 0:128], wtm[0])
            for m in range(1, 7):
                mw = 128 if m < 6 else 32
                nc.scalar.dma_start(wt_sb[:, :, m * 128:m * 128 + mw],
                                    wtm[m][:, :, 0:mw])
            wtv_bf = wtp.tile([128, nkc_hid, LKV], BF16)
            nc.scalar.dma_start(wtv_bf[:], wtvp[:])
            if bias:
                wtb_sb = wtp.tile([1, NQKV], BF16)
                nc.scalar.dma_start(wtb_sb[:], wtb[:])
                wtvb_bf = wtp.tile([1, LKV], BF16)
                nc.gpsimd.tensor_copy(wtvb_bf[:], wtb_sb[:, LQ + LKV:NQKV])

            if causal:
                mask_sb = persist.tile([128, QB // KC, QB], BF16)
                nc.scalar.dma_start(mask_sb[:],
                                    m01[:].rearrange("m p q -> p m q"))

            owt_sb = wtp.tile([128, LQ // 128, HID], BF16, name="owt_sb")
            nc.scalar.dma_start(
                owt_sb[:], owt[:].rearrange("(c p) j -> p c j", p=128))

            import contextlib
            loop_cm = tc.For_i(0, loop_n, 1) if loop_n else contextlib.nullcontext()
            with loop_cm:
              for _rep in range(rep):
                # ---------------- stage A: QKV projection ----------------
                def emit_a_units(tb):
                    """List of emit-callables: xtb load, 7 A1 m-chunks,
                    qk repack, 4 A2 (V) token groups."""
                    st = {}
                    units = []

                    def u_load():
                        xtb = xtp.tile([128, nkc_hid, TB], BF16, tag="xtb")
                        st["xtb"] = xtb
                        for c_ in range(nkc_hid):
                            nc.sync.dma_start(
                                xtb[:, c_, :],
                                xt_r[:, c_, tb * TB:(tb + 1) * TB])
                        if bias:
                            xb1 = xtp.tile([1, TB], BF16, tag="xb1")
                            nc.sync.dma_start(
                                xb1[:], xt[HID:HID + 1, tb * TB:(tb + 1) * TB])
                            st["xb1"] = xb1
                        qkpk_l[tb] = bwork.tile([128, 7, QB], BF16,
                                                name="qkpk", tag="qkpk",
                                                bufs=2)
                    units.append(u_load)

                    def mk_m(m):
                        def u_m():
                            mw = 128 if m < 6 else 32
                            ps = psA.tile([128, TB], F32, tag="a")
                            for c in range(nkc_a):
                                if c < nkc_hid:
                                    lhsT = wt_sb[:, c, m * 128:m * 128 + mw]
                                    rhs = st["xtb"][:, c, :]
                                else:
                                    lhsT = wtb_sb[:, m * 128:m * 128 + mw]
                                    rhs = st["xb1"][:]
                                nc.tensor.matmul(
                                    ps[0:mw, :], lhsT, rhs,
                                    start=(c == 0), stop=(c == nkc_a - 1),
                                )
                            nc.vector.tensor_copy(
                                qkpk_l[tb][0:mw, m, :], ps[0:mw, :])
                        return u_m
                    for m in range(7):
                        units.append(mk_m(m))

                    def u_repack():
                        qk_pk = qkpk_l[tb]
                        # k heads first: every score matmul of the next
                        # window needs k_l; q head h is only needed at h
                        for hh in list(range(HL, HL + KVL)) + list(range(HL)):
                            col0 = hh * 80
                            p0, c0 = col0 % 128, col0 // 128
                            n0 = min(80, 128 - p0)
                            dstt = (q_l[tb][:, hh, :] if hh < HL
                                    else k_l[tb][:, hh - HL, :])
                            nc.sync.dma_start(
                                dstt[0:n0, :], qk_pk[p0:p0 + n0, c0, :])
                            if n0 < 80:
                                nc.sync.dma_start(
                                    dstt[n0:80, :],
                                    qk_pk[0:80 - n0, c0 + 1, :])
                    units.append(u_repack)

                    def mk_a2(tci):
                        def u_a2():
                            tc_g = tb * (TB // 128) + tci
                            ps = psA.tile([128, LKV], F32, tag="a")
                            for c in range(nkc_a):
                                if c < nkc_hid:
                                    lhsT = st["xtb"][
                                        :, c, tci * 128:(tci + 1) * 128]
                                    rhs = wtv_bf[:, c, :]
                                else:
                                    lhsT = st["xb1"][
                                        :, tci * 128:(tci + 1) * 128]
                                    rhs = wtvb_bf[:]
                                nc.tensor.matmul(
                                    ps[:], lhsT, rhs,
                                    start=(c == 0), stop=(c == nkc_a - 1),
                                )
                            dst = v_l[tc_g // kc_per_qb][
                                :, tc_g % kc_per_qb, :].rearrange(
                                "p (kv e) -> p kv e", kv=2)[:, :, 0:HD]
                            src = ps[:].rearrange("p (kv e) -> p kv e", kv=2)
                            nc.vector.tensor_copy(dst, src)
                        return u_a2
                    for tci in range(TB // 128):
                        units.append(mk_a2(tci))
                    return units

                # ---------------- stage B: attention ---------------------
                b_state = {}

                def emit_b_head(qb, h):
                    if h == 0:
                        b_state["ow"] = bwork.tile([80, HL, QB], BF16,
                                                   name="owk", tag="ow",
                                                   bufs=2)
                    o_wk = b_state["ow"]
                    kv = h // G
                    nkc = (qb + 1) * kc_per_qb if causal else s // KC
                    ngrp = nkc // 2
                    pv = psPV.tile([97, QB], F32, tag="pv")
                    for g in range(ngrp):
                        def _qlo(kc):
                            return (max(0, kc * KC - qb * QB)
                                    if causal else 0)
                        gq = _qlo(2 * g)
                        sc = psSC.tile([128, 2, QB], F32, tag="sc")
                        for i in range(2):
                            kc = 2 * g + i
                            ql = _qlo(kc)
                            nc.tensor.matmul(
                                sc[:, i, ql:QB],
                                k_l[kc // kc_per_qb][
                                    :, kv,
                                    (kc % kc_per_qb) * KC:
                                    (kc % kc_per_qb + 1) * KC],
                                q_l[qb][:, h, ql:QB],
                                start=True, stop=True,
                            )
                        pt = bwork.tile([128, 2, QB], BF16, tag="pt",
                                        bufs=6)
                        nc.scalar.activation(
                            pt[:, :, gq:QB], sc[:, :, gq:QB],
                            mybir.ActivationFunctionType.Exp)
                        for i in range(2):
                            kc = 2 * g + i
                            ql = _qlo(kc)
                            if causal:
                                mi = kc - qb * kc_per_qb
                                if 0 <= mi < kc_per_qb:
                                    nc.vector.tensor_mul(
                                        pt[:, i, ql:QB], pt[:, i, ql:QB],
                                        mask_sb[:, mi, ql:QB],
                                    )
                            else:
                                mt = bwork.tile([128, QB], BF16, tag="mt",
                                                bufs=4)
                                nc.sync.dma_start(
                                    mt[:],
                                    m01[kc, :, qb * QB:(qb + 1) * QB])
                                nc.vector.tensor_mul(
                                    pt[:, i, :], pt[:, i, :], mt[:])
                            nc.tensor.matmul(
                                pv[0:97, ql:QB],
                                v_l[kc // kc_per_qb][
                                    :, kc % kc_per_qb,
                                    kv * VW:(kv + 1) * VW],
                                pt[:, i, ql:QB],
                                start=(kc == 0), stop=(kc == nkc - 1),
                                skip_group_check=True,
                            )
                    # normalize: o = pv[0:80] * (1/pv[96])
                    den = bwork.tile([1, QB], F32, tag="den")
                    nc.vector.tensor_copy(den[:], pv[96:97, :])
                    r_sb = bwork.tile([1, QB], F32, tag="r")
                    nc.vector.reciprocal_approx_fast(r_sb[:], den[:])
                    r_bc = bwork.tile([80, QB], F32, tag="rbc")
                    nc.gpsimd.partition_broadcast(r_bc[:], r_sb[:])
                    nc.vector.tensor_mul(
                        o_wk[:, h, :], pv[0:80, :], r_bc[:])
                    # pack this head into o_pk right away (hd = h*80+d) so
                    # the consuming D tiles unblock as soon as possible
                    hd0 = h * 80
                    p0, c0 = hd0 % 128, hd0 // 128
                    n0 = min(80, 128 - p0)
                    nc.sync.dma_start(
                        o_pk[qb][p0:p0 + n0, c0, :], o_wk[0:n0, h, :])
                    if n0 < 80:
                        nc.sync.dma_start(
                            o_pk[qb][0:80 - n0, c0 + 1, :],
                            o_wk[n0:80, h, :])

                # ---------------- stage D: o_proj partial -----------------
                def emit_d(tci):
                    jb, toff = tci // kc_per_qb, (tci % kc_per_qb) * 128
                    stg = dstage.tile([128, HID], F32, tag="stg")
                    nch = LQ // 128
                    for (j0, jn) in JBS:
                        ps = psA.tile([128, 512], F32, tag="a")
                        for c in range(nch):
                            nc.tensor.matmul(
                                ps[0:128, 0:jn],
                                o_pk[jb][:, c, toff:toff + 128],
                                owt_sb[:, c, j0:j0 + jn],
                                start=(c == 0), stop=(c == nch - 1),
                            )
                        nc.vector.tensor_copy(stg[:, j0:j0 + jn],
                                              ps[0:128, 0:jn])
                        nc.sync.dma_start(
                            out[tci * 128:(tci + 1) * 128, j0:j0 + jn],
                            stg[:, j0:j0 + jn])

                if causal:
                    # interleaved schedule: stage-B heads with A/D fillers
                    for u in emit_a_units(0):
                        u()
                    for qb in range(nqb):
                        if qb == nqb - 1:
                            fillers = [
                                (lambda t=t: emit_d(t))
                                for t in range(4, 12)
                            ]
                        elif qb == nqb - 2:
                            fillers = emit_a_units(qb + 1) + [
                                (lambda t=t: emit_d(t)) for t in range(0, 4)
                            ]
                        else:
                            fillers = emit_a_units(qb + 1)
                        nu, pos = len(fillers), 0
                        for h in range(HL):
                            emit_b_head(qb, h)
                            nxt = ((h + 1) * nu) // HL
                            while pos < nxt:
                                fillers[pos]()
                                pos += 1
                    for tci in range(12, 16):
                        emit_d(tci)
                else:
                    for tb in range(ntb):
                        for u in emit_a_units(tb):
                            u()
                    for qb in range(nqb):
                        for h in range(HL):
                            emit_b_head(qb, h)
                    for tci in range(n_tc):
                        emit_d(tci)
    nc.compile()
    return nc


# ---------------------------------------------------------------------------
# cached PJRT runner (replica of bass2jax.run_bass_via_pjrt with jit reuse)
# ---------------------------------------------------------------------------
@functools.lru_cache(maxsize=4)
def _get_runner(s, causal, bias, rep=1, loop_n=0):
    import jax
    import jax.numpy as jnp
    from jax.sharding import Mesh, PartitionSpec
    from jax.experimental.shard_map import shard_map
    from concourse import bass2jax
    from concourse import mybir as _mybir

    nc = _build(s, causal, bias, rep, loop_n)
    bass2jax.install_neuronx_cc_hook()

    partition_name = (
        nc.partition_id_tensor.name if nc.partition_id_tensor else None
    )
    in_names, out_names, out_avals, zero_outs = [], [], [], []
    for alloc in nc.m.functions[0].allocations:
        if not isinstance(alloc, _mybir.MemoryLocationSet):
            continue
        name = alloc.memorylocations[0].name
        if alloc.kind == "ExternalInput":
            if name != partition_name:
                in_names.append(name)
        elif alloc.kind == "ExternalOutput":
            shape = tuple(alloc.tensor_shape)
            dtype = _mybir.dt.np(alloc.dtype)
            out_names.append(name)
            out_avals.append(jax.core.ShapedArray(shape, dtype))
            zero_outs.append(np.zeros(shape, dtype))
    n_params = len(in_names)
    n_outs = len(out_avals)
    all_names = in_names + out_names
    if partition_name is not None:
        all_names = all_names + [partition_name]

    def _body(*args):
        operands = list(args)
        if partition_name is not None:
            operands.append(bass2jax.partition_id_tensor())
        outs = bass2jax._bass_exec_p.bind(
            *operands,
            out_avals=tuple(out_avals),
            in_names=tuple(all_names),
            out_names=tuple(out_names),
            lowering_input_output_aliases=(),
            sim_require_finite=True,
            sim_require_nnan=True,
            nc=nc,
        )
        return tuple(outs)

    devices = jax.devices()[:NCORE]
    mesh = Mesh(np.asarray(devices), ("core",))
    donate = tuple(range(n_params, n_params + n_outs))
    sharded = jax.jit(
        shard_map(
            _body, mesh=mesh,
            in_specs=(PartitionSpec("core"),) * (n_params + n_outs),
            out_specs=(PartitionSpec("core"),) * n_outs,
            check_rep=False,
        ),
        donate_argnums=donate,
        keep_unused=True,
    )

    def run(in_maps):
        from jax.sharding import NamedSharding
        sh = NamedSharding(mesh, PartitionSpec("core"))
        concat_in = [
            np.concatenate([np.asarray(m[name]) for m in in_maps], axis=0)
            for name in in_names
        ]
        concat_zeros = [
            jnp.zeros((NCORE * z.shape[0], *z.shape[1:]), z.dtype, device=sh)
            for z in zero_outs
        ]
        out_arrs = sharded(*concat_in, *concat_zeros)
        return [
            {
                name: np.asarray(out_arrs[i]).reshape(
                    NCORE, *out_avals[i].shape)[c]
                for i, name in enumerate(out_names)
            }
            for c in range(NCORE)
        ]

    def bench(in_maps, iters=10):
        """Time device execution with device-resident inputs, no donation."""
        from jax.sharding import NamedSharding
        import time as _time

        nodonate = jax.jit(
            shard_map(
                _body, mesh=mesh,
                in_specs=(PartitionSpec("core"),) * (n_params + n_outs),
                out_specs=(PartitionSpec("core"),) * n_outs,
                check_rep=False,
            ),
            keep_unused=True,
        )
        sh = NamedSharding(mesh, PartitionSpec("core"))
        dev_in = [
            jax.device_put(
                np.concatenate([np.asarray(m[name]) for m in in_maps], axis=0),
                sh)
            for name in in_names
        ]
        dev_zeros = [
            jax.device_put(
                np.zeros((NCORE * z.shape[0], *z.shape[1:]), z.dtype), sh)
            for z in zero_outs
        ]
        out = nodonate(*dev_in, *dev_zeros)
        jax.block_until_ready(out)
        times = []
        for _ in range(iters):
            t0 = _time.perf_counter()
            out = nodonate(*dev_in, *dev_zeros)
            jax.block_until_ready(out)
            times.append(_time.perf_counter() - t0)
        return times

    def bench_chain(in_maps, chain, iters=5):
        """Chain `chain` kernel executions in one dispatch (output buffer of
        call i feeds call i+1 as the to-be-overwritten out buffer), so the
        ~70ms axon dispatch overhead amortizes. Returns list of wall times."""
        from jax.sharding import NamedSharding
        import time as _time

        assert n_outs == 1

        def _chained(*args):
            ins, out = list(args[:n_params]), args[n_params]
            for _ in range(chain):
                out = _body(*ins, out)[0]
            return out

        f = jax.jit(
            shard_map(
                _chained, mesh=mesh,
                in_specs=(PartitionSpec("core"),) * (n_params + 1),
                out_specs=PartitionSpec("core"),
                check_rep=False,
            ),
            keep_unused=True,
        )
        sh = NamedSharding(mesh, PartitionSpec("core"))
        dev_in = [
            jax.device_put(
                np.concatenate([np.asarray(m[name]) for m in in_maps], axis=0),
                sh)
            for name in in_names
        ]
        z = zero_outs[0]
        dev_zero = jax.device_put(
            np.zeros((NCORE * z.shape[0], *z.shape[1:]), z.dtype), sh)
        jax.block_until_ready(f(*dev_in, dev_zero))
        times = []
        for _ in range(iters):
            t0 = _time.perf_counter()
            jax.block_until_ready(f(*dev_in, dev_zero))
            times.append(_time.perf_counter() - t0)
        return times

    run.bench = bench
    run.bench_chain = bench_chain
    return run


# ---------------------------------------------------------------------------
# host wrapper
# ---------------------------------------------------------------------------
def _softplus(x):
    return np.logaddexp(0.0, x).astype(np.float32)


def _causal_mask_tiles():
    kk = np.arange(KC)[:, None]
    qq = np.arange(QB)[None, :]
    tiles = np.stack(
        [(qq >= kk + m * KC) for m in range(QB // KC)]
    ).astype(ml_dtypes.bfloat16)
    return tiles  # [4, 128, 512]


def _is_causal(mask, neg=-2.3819763e38):
    m = mask.reshape(mask.shape[-2], mask.shape[-1])
    expect = np.where(
        np.tril(np.ones(m.shape, dtype=bool)), np.float32(0.0), np.float32(neg)
    )
    return np.array_equal(m, expect)


def prepare_inputs(hidden_states, mask, scaling, qkv_w, qkv_b, o_w, o_b):
    s = hidden_states.shape[1]
    hidden_states = np.asarray(hidden_states, dtype=np.float32)
    mask = np.asarray(mask, dtype=np.float32)
    scaling = np.asarray(scaling, dtype=np.float32)
    qkv_w = np.asarray(qkv_w, dtype=np.float32)
    qkv_b = np.asarray(qkv_b, dtype=np.float32)
    o_w = np.asarray(o_w, dtype=np.float32)
    o_b = np.asarray(o_b, dtype=np.float32)

    causal = bool(_is_causal(mask))
    bias = bool(np.any(qkv_b))

    scale = (1.442695041 / math.sqrt(HD)) * _softplus(scaling)  # [80]
    wq = qkv_w[:Q_SIZE] * np.tile(scale, NH)[:, None]           # scaled
    bq = qkv_b[:Q_SIZE] * np.tile(scale, NH)

    if causal:
        m01_full = _causal_mask_tiles()
    else:
        # exp(mask) transposed to [k, q], tiled as [s/128, 128, s]
        me = np.exp(mask.reshape(s, s).T.astype(np.float32))
        m01_full = np.ascontiguousarray(
            me.reshape(s // KC, KC, s)).astype(ml_dtypes.bfloat16)

    # xt depends only on batch; wt/owt only on head-group -> build each once
    xts = []
    hs_bf = hidden_states.astype(ml_dtypes.bfloat16)
    for b in range(B):
        xt = np.ascontiguousarray(hs_bf[b].T)                  # [1280, s] bf16
        if bias:
            xt = np.concatenate(
                [xt, np.ones((1, s), ml_dtypes.bfloat16)], axis=0)
        xts.append(xt)
    wtms, wtvps, wtbs, owts = [], [], [], []
    for hg in range(2):
        qrows = slice(hg * LQ, (hg + 1) * LQ)
        krows = slice(Q_SIZE + hg * LKV, Q_SIZE + (hg + 1) * LKV)
        vrows = slice(Q_SIZE + KV_SIZE + hg * LKV,
                      Q_SIZE + KV_SIZE + (hg + 1) * LKV)
        w_slice = np.concatenate(
            [wq[qrows], qkv_w[krows], qkv_w[vrows]], axis=0)   # [960, 1280]
        wtT = np.ascontiguousarray(w_slice.T)                  # [1280, 960]
        arr = wtT.reshape(HID // 128, 128, NQKV)               # [c, p, n]
        # m-major prepack: wtm[m, p, c, j] = wtT[c*128+p, m*128+j]
        wtm = np.zeros((7, 128, HID // 128, 128), dtype=np.float32)
        for m in range(7):
            mw = 128 if m < 6 else 32
            wtm[m, :, :, 0:mw] = arr[:, :, m * 128:m * 128 + mw].transpose(
                1, 0, 2)
        wtms.append(np.ascontiguousarray(wtm).astype(ml_dtypes.bfloat16))
        wtvps.append(np.ascontiguousarray(
            arr[:, :, LQ + LKV:NQKV].transpose(1, 0, 2)).astype(
            ml_dtypes.bfloat16))
        if bias:
            b_slice = np.concatenate([bq[qrows], qkv_b[krows], qkv_b[vrows]])
            wtbs.append(b_slice[None, :].astype(ml_dtypes.bfloat16))
        owts.append(np.ascontiguousarray(
            o_w[:, hg * LQ:(hg + 1) * LQ].T).astype(ml_dtypes.bfloat16))
    in_maps = []
    for c in range(NCORE):
        b, hg = divmod(c, 2)
        m = {"xt": xts[b], "wtm": wtms[hg], "wtvp": wtvps[hg],
             "owt": owts[hg], "m01": m01_full}
        if bias:
            m["wtb"] = wtbs[hg]
        in_maps.append(m)
    return in_maps, causal, bias, o_b


def kernel(hidden_states, mask, scaling, qkv_w, qkv_b, o_w, o_b):
    s = hidden_states.shape[1]
    in_maps, causal, bias, o_b32 = prepare_inputs(
        hidden_states, mask, scaling, qkv_w, qkv_b, o_w, o_b)
    run = _get_runner(s, causal, bias)
    res = run(in_maps)
    out = np.empty((B, s, HID), dtype=np.float32)
    for b in range(B):
        out[b] = res[2 * b]["out"] + res[2 * b + 1]["out"] + o_b32[None, :]
    return out



# revision 45
# speedup vs baseline: 1.3992x; 1.3992x over previous
"""Trainium2 Bass kernel for GQA multi-head attention (B=4, S=2048, HID=1280,
NH=16, NKV=4, HD=80) sharded over 8 NeuronCores as (batch x kv-head-group).

Per core (b, hg): 8 q heads / 2 kv heads of batch b.
  A1: Q/K projection, d-major bf16 m-chunks (K chunks first so the k_l
      repack lands early); host-prepacked m-major weights load at full DMA
      bandwidth.  A2: V projection token-major + ones column.
  B:  scores (bf16) -> exp (ACT) -> causal 0/1 mask mul (DVE)
      -> PV matmul with ones-row denominator -> normalize (recip+bcast+mul),
      per-head o_pk pack right after normalize.
  D:  o_proj row-parallel partial, bf16 output; host sums the two head-group
      partials in f32.

Schedule: stage B is ACT(exp)-bound, so A/D matmul "filler units" are
interleaved between attention heads to keep the PE busy; two D fillers are
reserved for after the last head to cover its normalize+pack latency.  DMA
queues: loads/repacks/packs/stores on the sync HWDGE queue, weight preloads
on the scalar HWDGE queue, nothing bulky on the Pool SWDGE queue (software
descriptor generation there would stall partition_broadcast).  The tb=0 x
block lives in a persistent tile reloaded at each loop-body tail so the next
iteration's first matmuls start immediately.
"""

import functools
import math

import numpy as np
import ml_dtypes

import concourse.bass as bass
import concourse.mybir as mybir
import concourse.tile as tile
from concourse import bacc

B, S, HID = 4, 2048, 1280
NH, NKV, HD = 16, 4, 80
G = NH // NKV  # 4
Q_SIZE, KV_SIZE = NH * HD, NKV * HD
NCORE = 8
HL = 8          # local q heads per core
KVL = 2         # local kv heads per core
LQ = HL * HD    # 640 local q cols
LKV = KVL * HD  # 160 local k (and v) cols
NQKV = LQ + 2 * LKV  # 960 local qkv cols

F32 = mybir.dt.float32
F32R = mybir.dt.float32r
BF16 = mybir.dt.bfloat16

TB = 512        # stage-A token block
QB = 512        # stage-B q block
KC = 128        # k chunk (partitions)


def _build(s, causal, bias, rep=1, loop_n=0):
    """Build + compile the per-core Bass program. Same program on all cores.

    v2 schedule: stage-B (attention) is ACT-bound (exp); stage-A/D matmul
    "filler units" are interleaved between attention heads so the PE never
    idles waiting on the exp pipeline.  Loads/repacks/packs/stores ride the
    sync (SP) HWDGE queue; weight preloads ride the scalar HWDGE queue.
    """
    nqb = s // QB
    ntb = s // TB
    nkc_hid = HID // KC  # 10
    n_tc = s // 128
    tb_per_qb = QB // TB   # 1
    kc_per_qb = QB // KC   # 4
    assert TB == QB

    nc = bacc.Bacc(None)
    xt = nc.declare_dram_parameter("xt", [HID + (1 if bias else 0), s], BF16,
                                   isOutput=False)
    # wtm: host-prepacked m-major qk weight chunks [m, p, c, j] so each
    # m-chunk loads as one full-bandwidth DMA (contiguous per partition)
    wtm = nc.declare_dram_parameter("wtm", [7, 128, HID // KC, 128], BF16,
                                    isOutput=False)
    wtvp = nc.declare_dram_parameter("wtvp", [128, HID // KC, LKV], BF16,
                                     isOutput=False)
    if bias:
        wtb = nc.declare_dram_parameter("wtb", [1, NQKV], BF16,
                                        isOutput=False)
    owt = nc.declare_dram_parameter("owt", [LQ, HID], BF16, isOutput=False)
    if causal:
        m01 = nc.declare_dram_parameter("m01", [QB // KC, KC, QB], BF16,
                                        isOutput=False)
    else:
        m01 = nc.declare_dram_parameter("m01", [s // KC, KC, s], BF16,
                                        isOutput=False)
    out = nc.declare_dram_parameter("out", [s, HID], BF16, isOutput=True)

    nkc_a = nkc_hid + (1 if bias else 0)
    xt_r = xt[0:HID, :].rearrange("(c p) t -> p c t", p=128)
    VW = 97  # 80 v cols + 16 zero pad + ones col at 96
    JBS = [(0, 512), (512, 512), (1024, 256)]

    with tile.TileContext(nc) as tc:
        with (
            tc.tile_pool(name="persist", bufs=1) as persist,
            tc.tile_pool(name="wtp", bufs=1) as wtp,
            tc.tile_pool(name="xtp", bufs=2) as xtp,
            tc.tile_pool(name="bwork", bufs=2) as bwork,
            tc.tile_pool(name="dstage", bufs=2) as dstage,
            tc.tile_pool(name="psA", bufs=2, space="PSUM") as psA,
            tc.tile_pool(name="psSC", bufs=2, space="PSUM") as psSC,
            tc.tile_pool(name="psPV", bufs=2, space="PSUM") as psPV,
        ):
            # ---- persistent SBUF, split per qb-block for fine-grained deps
            q_l = [persist.tile([80, HL, QB], BF16, name=f"q{j}")
                   for j in range(nqb)]
            k_l = [persist.tile([80, KVL, QB], BF16, name=f"k{j}")
                   for j in range(nqb)]
            v_l = [persist.tile([128, kc_per_qb, 2 * VW], BF16, name=f"v{j}")
                   for j in range(nqb)]
            o_pk = [[persist.tile([128, QB], BF16, name=f"opk{j}_{c}")
                     for c in range(LQ // 128)] for j in range(nqb)]
            xtb0 = persist.tile([128, nkc_hid, TB], BF16, name="xtb0")
            qkpk_l = [None] * nqb
            for j in range(nqb):
                nc.vector.memset(v_l[j][:], 0.0)
                nc.vector.memset(v_l[j][:, :, 96:97], 1.0)
                nc.vector.memset(v_l[j][:, :, VW + 96:VW + 97], 1.0)

            # ---- stage A weights.  m-chunk 0 goes first on the sync queue
            # (the first A1 matmul needs it + the first xtb block); the rest
            # stream on the scalar-engine HWDGE queue in parallel.  mask/owt
            # (needed much later) are emitted last so they queue behind the
            # startup-critical loads on the shared DMA engines.
            wt_sb = wtp.tile([128, nkc_hid, LQ + LKV], BF16)
            nc.sync.dma_start(wt_sb[:, :, 5 * 128:6 * 128], wtm[5])
            # prologue load of the tb=0 x block; each loop body re-loads it
            # at its tail (off the critical path) so the next iteration's
            # first matmuls find it resident
            for c_ in range(nkc_hid):
                nc.sync.dma_start(xtb0[:, c_, :], xt_r[:, c_, 0:TB])
            for m in (6, 0, 1, 2, 3, 4):
                mw = 128 if m < 6 else 32
                nc.scalar.dma_start(wt_sb[:, :, m * 128:m * 128 + mw],
                                    wtm[m][:, :, 0:mw])
            wtv_bf = wtp.tile([128, nkc_hid, LKV], BF16)
            nc.scalar.dma_start(wtv_bf[:], wtvp[:])
            if bias:
                wtb_sb = wtp.tile([1, NQKV], BF16)
                nc.scalar.dma_start(wtb_sb[:], wtb[:])
                wtvb_bf = wtp.tile([1, LKV], BF16)
                nc.gpsimd.tensor_copy(wtvb_bf[:], wtb_sb[:, LQ + LKV:NQKV])

            if causal:
                mask_sb = persist.tile([128, QB // KC, QB], BF16)
                nc.scalar.dma_start(mask_sb[:],
                                    m01[:].rearrange("m p q -> p m q"))

            owt_sb = wtp.tile([128, LQ // 128, HID], BF16, name="owt_sb")
            nc.scalar.dma_start(
                owt_sb[:], owt[:].rearrange("(c p) j -> p c j", p=128))

            import contextlib
            loop_cm = tc.For_i(0, loop_n, 1) if loop_n else contextlib.nullcontext()
            with loop_cm:
              for _rep in range(rep):
                # ---------------- stage A: QKV projection ----------------
                def emit_a_units(tb):
                    """List of emit-callables: xtb load, 7 A1 m-chunks,
                    qk repack, 4 A2 (V) token groups."""
                    st = {}
                    units = []

                    def u_load():
                        if tb == 0:
                            st["xtb"] = xtb0
                        else:
                            xtb = xtp.tile([128, nkc_hid, TB], BF16,
                                           tag="xtb")
                            st["xtb"] = xtb
                            for c_ in range(nkc_hid):
                                nc.sync.dma_start(
                                    xtb[:, c_, :],
                                    xt_r[:, c_, tb * TB:(tb + 1) * TB])
                        if bias:
                            xb1 = xtp.tile([1, TB], BF16, tag="xb1")
                            nc.sync.dma_start(
                                xb1[:], xt[HID:HID + 1, tb * TB:(tb + 1) * TB])
                            st["xb1"] = xb1
                        qkpk_l[tb] = bwork.tile([128, 7, QB], BF16,
                                                name="qkpk", tag="qkpk",
                                                bufs=2)
                    units.append(u_load)

                    def mk_m(m):
                        def u_m():
                            mw = 128 if m < 6 else 32
                            ps = psA.tile([128, TB], F32, tag="a")
                            for c in range(nkc_a):
                                if c < nkc_hid:
                                    lhsT = wt_sb[:, c, m * 128:m * 128 + mw]
                                    rhs = st["xtb"][:, c, :]
                                else:
                                    lhsT = wtb_sb[:, m * 128:m * 128 + mw]
                                    rhs = st["xb1"][:]
                                nc.tensor.matmul(
                                    ps[0:mw, :], lhsT, rhs,
                                    start=(c == 0), stop=(c == nkc_a - 1),
                                )
                            nc.vector.tensor_copy(
                                qkpk_l[tb][0:mw, m, :], ps[0:mw, :])
                        return u_m

                    def mk_repack(heads):
                        def u_rp():
                            qk_pk = qkpk_l[tb]
                            for hh in heads:
                                col0 = hh * 80
                                p0, c0 = col0 % 128, col0 // 128
                                n0 = min(80, 128 - p0)
                                dstt = (q_l[tb][:, hh, :] if hh < HL
                                        else k_l[tb][:, hh - HL, :])
                                nc.sync.dma_start(
                                    dstt[0:n0, :], qk_pk[p0:p0 + n0, c0, :])
                                if n0 < 80:
                                    nc.sync.dma_start(
                                        dstt[n0:80, :],
                                        qk_pk[0:80 - n0, c0 + 1, :])
                        return u_rp

                    # K chunks (m5, m6) first so k_l repacks early — every
                    # head of the next window's scores needs k_l; q head h
                    # repacks right after the last m-chunk it spans.
                    units.append(mk_m(5))
                    units.append(mk_m(6))
                    units.append(mk_repack([HL, HL + 1]))
                    units.append(mk_m(0))
                    units.append(mk_repack([0]))
                    units.append(mk_m(1))
                    units.append(mk_repack([1, 2]))
                    units.append(mk_m(2))
                    units.append(mk_repack([3]))
                    units.append(mk_m(3))
                    units.append(mk_repack([4, 5]))
                    units.append(mk_m(4))
                    units.append(mk_repack([6, 7]))

                    def mk_a2(tci):
                        def u_a2():
                            tc_g = tb * (TB // 128) + tci
                            ps = psA.tile([128, LKV], F32, tag="a")
                            for c in range(nkc_a):
                                if c < nkc_hid:
                                    lhsT = st["xtb"][
                                        :, c, tci * 128:(tci + 1) * 128]
                                    rhs = wtv_bf[:, c, :]
                                else:
                                    lhsT = st["xb1"][
                                        :, tci * 128:(tci + 1) * 128]
                                    rhs = wtvb_bf[:]
                                nc.tensor.matmul(
                                    ps[:], lhsT, rhs,
                                    start=(c == 0), stop=(c == nkc_a - 1),
                                )
                            dst = v_l[tc_g // kc_per_qb][
                                :, tc_g % kc_per_qb, :].rearrange(
                                "p (kv e) -> p kv e", kv=2)[:, :, 0:HD]
                            src = ps[:].rearrange("p (kv e) -> p kv e", kv=2)
                            nc.vector.tensor_copy(dst, src)
                        return u_a2
                    for tci in range(TB // 128):
                        units.append(mk_a2(tci))
                    return units

                # ---------------- stage B: attention ---------------------
                b_state = {}

                def emit_b_head(qb, h):
                    if h == 0:
                        b_state["ow"] = bwork.tile([80, HL, QB], BF16,
                                                   name="owk", tag="ow",
                                                   bufs=2)
                    o_wk = b_state["ow"]
                    kv = h // G
                    nkc = (qb + 1) * kc_per_qb if causal else s // KC
                    ngrp = nkc // 2
                    pv = psPV.tile([97, QB], F32, tag="pv")
                    for g in range(ngrp):
                        def _qlo(kc):
                            return (max(0, kc * KC - qb * QB)
                                    if causal else 0)
                        gq = _qlo(2 * g)
                        sc = psSC.tile([128, 2, QB], F32, tag="sc")
                        for i in range(2):
                            kc = 2 * g + i
                            ql = _qlo(kc)
                            nc.tensor.matmul(
                                sc[:, i, ql:QB],
                                k_l[kc // kc_per_qb][
                                    :, kv,
                                    (kc % kc_per_qb) * KC:
                                    (kc % kc_per_qb + 1) * KC],
                                q_l[qb][:, h, ql:QB],
                                start=True, stop=True,
                            )
                        pt = bwork.tile([128, 2, QB], BF16, tag="pt",
                                        bufs=6)
                        nc.scalar.activation(
                            pt[:, :, gq:QB], sc[:, :, gq:QB],
                            mybir.ActivationFunctionType.Exp)
                        for i in range(2):
                            kc = 2 * g + i
                            ql = _qlo(kc)
                            if causal:
                                mi = kc - qb * kc_per_qb
                                if 0 <= mi < kc_per_qb:
                                    nc.vector.tensor_mul(
                                        pt[:, i, ql:QB], pt[:, i, ql:QB],
                                        mask_sb[:, mi, ql:QB],
                                    )
                            else:
                                mt = bwork.tile([128, QB], BF16, tag="mt",
                                                bufs=4)
                                nc.sync.dma_start(
                                    mt[:],
                                    m01[kc, :, qb * QB:(qb + 1) * QB])
                                nc.vector.tensor_mul(
                                    pt[:, i, :], pt[:, i, :], mt[:])
                            nc.tensor.matmul(
                                pv[0:97, ql:QB],
                                v_l[kc // kc_per_qb][
                                    :, kc % kc_per_qb,
                                    kv * VW:(kv + 1) * VW],
                                pt[:, i, ql:QB],
                                start=(kc == 0), stop=(kc == nkc - 1),
                                skip_group_check=True,
                            )
                    # normalize: o = pv[0:80] * (1/pv[96])
                    den = bwork.tile([1, QB], F32, tag="den")
                    nc.vector.tensor_copy(den[:], pv[96:97, :])
                    r_sb = bwork.tile([1, QB], F32, tag="r")
                    nc.vector.reciprocal_approx_fast(r_sb[:], den[:])
                    r_bc = bwork.tile([80, QB], F32, tag="rbc")
                    nc.gpsimd.partition_broadcast(r_bc[:], r_sb[:])
                    nc.vector.tensor_mul(
                        o_wk[:, h, :], pv[0:80, :], r_bc[:])
                    # pack this head into o_pk right away (hd = h*80+d) so
                    # the consuming D tiles unblock as soon as possible
                    hd0 = h * 80
                    p0, c0 = hd0 % 128, hd0 // 128
                    n0 = min(80, 128 - p0)
                    nc.sync.dma_start(
                        o_pk[qb][c0][p0:p0 + n0, :], o_wk[0:n0, h, :])
                    if n0 < 80:
                        nc.sync.dma_start(
                            o_pk[qb][c0 + 1][0:80 - n0, :],
                            o_wk[n0:80, h, :])

                # ---------------- stage D: o_proj partial -----------------
                def emit_d(tci):
                    jb, toff = tci // kc_per_qb, (tci % kc_per_qb) * 128
                    stg = dstage.tile([128, HID], BF16, tag="stg")
                    nch = LQ // 128
                    # first two 512-col groups c-outer so each o_pk chunk's
                    # weight load is shared by both matmuls
                    ps0 = psA.tile([128, 512], F32, tag="a")
                    ps1 = psA.tile([128, 512], F32, tag="a")
                    for c in range(nch):
                        for (j0, ps) in ((0, ps0), (512, ps1)):
                            nc.tensor.matmul(
                                ps[0:128, 0:512],
                                o_pk[jb][c][:, toff:toff + 128],
                                owt_sb[:, c, j0:j0 + 512],
                                start=(c == 0), stop=(c == nch - 1),
                                skip_group_check=True,
                            )
                    for (j0, ps) in ((0, ps0), (512, ps1)):
                        nc.vector.tensor_copy(stg[:, j0:j0 + 512],
                                              ps[0:128, 0:512])
                        nc.sync.dma_start(
                            out[tci * 128:(tci + 1) * 128, j0:j0 + 512],
                            stg[:, j0:j0 + 512])
                    ps = psA.tile([128, 512], F32, tag="a")
                    for c in range(nch):
                        nc.tensor.matmul(
                            ps[0:128, 0:256],
                            o_pk[jb][c][:, toff:toff + 128],
                            owt_sb[:, c, 1024:1280],
                            start=(c == 0), stop=(c == nch - 1),
                        )
                    nc.vector.tensor_copy(stg[:, 1024:1280],
                                          ps[0:128, 0:256])
                    nc.sync.dma_start(
                        out[tci * 128:(tci + 1) * 128, 1024:1280],
                        stg[:, 1024:1280])

                if causal:
                    # interleaved schedule: stage-B heads with A/D fillers
                    for u in emit_a_units(0):
                        u()
                    for qb in range(nqb):
                        if qb == nqb - 1:
                            fillers = [
                                (lambda t=t: emit_d(t))
                                for t in range(4, 12)
                            ]
                        elif qb == nqb - 2:
                            fillers = emit_a_units(qb + 1) + [
                                (lambda t=t: emit_d(t)) for t in range(0, 4)
                            ]
                        else:
                            fillers = emit_a_units(qb + 1)
                        nu, pos = len(fillers), 0
                        last_w = qb == nqb - 1
                        for h in range(HL):
                            emit_b_head(qb, h)
                            if last_w:
                                # keep two fillers for after the last head so
                                # its normalize+pack latency is covered
                                nxt = (nu if h == HL - 1
                                       else ((h + 1) * (nu - 2)) // (HL - 1))
                            else:
                                nxt = ((h + 1) * nu) // HL
                            while pos < nxt:
                                fillers[pos]()
                                pos += 1
                    nc.scalar.dma_start(xtb0[:], xt_r[:, :, 0:TB])
                    for tci in range(12, 16):
                        emit_d(tci)
                else:
                    for tb in range(ntb):
                        for u in emit_a_units(tb):
                            u()
                    for qb in range(nqb):
                        for h in range(HL):
                            emit_b_head(qb, h)
                    nc.scalar.dma_start(xtb0[:], xt_r[:, :, 0:TB])
                    for tci in range(n_tc):
                        emit_d(tci)
    nc.compile()
    return nc


# ---------------------------------------------------------------------------
# cached PJRT runner (replica of bass2jax.run_bass_via_pjrt with jit reuse)
# ---------------------------------------------------------------------------
@functools.lru_cache(maxsize=4)
def _get_runner(s, causal, bias, rep=1, loop_n=0):
    import jax
    import jax.numpy as jnp
    from jax.sharding import Mesh, PartitionSpec
    from jax.experimental.shard_map import shard_map
    from concourse import bass2jax
    from concourse import mybir as _mybir

    nc = _build(s, causal, bias, rep, loop_n)
    bass2jax.install_neuronx_cc_hook()

    partition_name = (
        nc.partition_id_tensor.name if nc.partition_id_tensor else None
    )
    in_names, out_names, out_avals, zero_outs = [], [], [], []
    for alloc in nc.m.functions[0].allocations:
        if not isinstance(alloc, _mybir.MemoryLocationSet):
            continue
        name = alloc.memorylocations[0].name
        if alloc.kind == "ExternalInput":
            if name != partition_name:
                in_names.append(name)
        elif alloc.kind == "ExternalOutput":
            shape = tuple(alloc.tensor_shape)
            dtype = _mybir.dt.np(alloc.dtype)
            out_names.append(name)
            out_avals.append(jax.core.ShapedArray(shape, dtype))
            zero_outs.append(np.zeros(shape, dtype))
    n_params = len(in_names)
    n_outs = len(out_avals)
    all_names = in_names + out_names
    if partition_name is not None:
        all_names = all_names + [partition_name]

    def _body(*args):
        operands = list(args)
        if partition_name is not None:
            operands.append(bass2jax.partition_id_tensor())
        outs = bass2jax._bass_exec_p.bind(
            *operands,
            out_avals=tuple(out_avals),
            in_names=tuple(all_names),
            out_names=tuple(out_names),
            lowering_input_output_aliases=(),
            sim_require_finite=True,
            sim_require_nnan=True,
            nc=nc,
        )
        return tuple(outs)

    devices = jax.devices()[:NCORE]
    mesh = Mesh(np.asarray(devices), ("core",))
    donate = tuple(range(n_params, n_params + n_outs))
    sharded = jax.jit(
        shard_map(
            _body, mesh=mesh,
            in_specs=(PartitionSpec("core"),) * (n_params + n_outs),
            out_specs=(PartitionSpec("core"),) * n_outs,
            check_rep=False,
        ),
        donate_argnums=donate,
        keep_unused=True,
    )

    def run(in_maps):
        from jax.sharding import NamedSharding
        sh = NamedSharding(mesh, PartitionSpec("core"))
        concat_in = [
            np.concatenate([np.asarray(m[name]) for m in in_maps], axis=0)
            for name in in_names
        ]
        concat_zeros = [
            jnp.zeros((NCORE * z.shape[0], *z.shape[1:]), z.dtype, device=sh)
            for z in zero_outs
        ]
        out_arrs = sharded(*concat_in, *concat_zeros)
        return [
            {
                name: np.asarray(out_arrs[i]).reshape(
                    NCORE, *out_avals[i].shape)[c]
                for i, name in enumerate(out_names)
            }
            for c in range(NCORE)
        ]

    def bench(in_maps, iters=10):
        """Time device execution with device-resident inputs, no donation."""
        from jax.sharding import NamedSharding
        import time as _time

        nodonate = jax.jit(
            shard_map(
                _body, mesh=mesh,
                in_specs=(PartitionSpec("core"),) * (n_params + n_outs),
                out_specs=(PartitionSpec("core"),) * n_outs,
                check_rep=False,
            ),
            keep_unused=True,
        )
        sh = NamedSharding(mesh, PartitionSpec("core"))
        dev_in = [
            jax.device_put(
                np.concatenate([np.asarray(m[name]) for m in in_maps], axis=0),
                sh)
            for name in in_names
        ]
        dev_zeros = [
            jax.device_put(
                np.zeros((NCORE * z.shape[0], *z.shape[1:]), z.dtype), sh)
            for z in zero_outs
        ]
        out = nodonate(*dev_in, *dev_zeros)
        jax.block_until_ready(out)
        times = []
        for _ in range(iters):
            t0 = _time.perf_counter()
            out = nodonate(*dev_in, *dev_zeros)
            jax.block_until_ready(out)
            times.append(_time.perf_counter() - t0)
        return times

    def bench_chain(in_maps, chain, iters=5):
        """Chain `chain` kernel executions in one dispatch (output buffer of
        call i feeds call i+1 as the to-be-overwritten out buffer), so the
        ~70ms axon dispatch overhead amortizes. Returns list of wall times."""
        from jax.sharding import NamedSharding
        import time as _time

        assert n_outs == 1

        def _chained(*args):
            ins, out = list(args[:n_params]), args[n_params]
            for _ in range(chain):
                out = _body(*ins, out)[0]
            return out

        f = jax.jit(
            shard_map(
                _chained, mesh=mesh,
                in_specs=(PartitionSpec("core"),) * (n_params + 1),
                out_specs=PartitionSpec("core"),
                check_rep=False,
            ),
            keep_unused=True,
        )
        sh = NamedSharding(mesh, PartitionSpec("core"))
        dev_in = [
            jax.device_put(
                np.concatenate([np.asarray(m[name]) for m in in_maps], axis=0),
                sh)
            for name in in_names
        ]
        z = zero_outs[0]
        dev_zero = jax.device_put(
            np.zeros((NCORE * z.shape[0], *z.shape[1:]), z.dtype), sh)
        jax.block_until_ready(f(*dev_in, dev_zero))
        times = []
        for _ in range(iters):
            t0 = _time.perf_counter()
            jax.block_until_ready(f(*dev_in, dev_zero))
            times.append(_time.perf_counter() - t0)
        return times

    run.bench = bench
    run.bench_chain = bench_chain
    return run


# ---------------------------------------------------------------------------
# host wrapper
# ---------------------------------------------------------------------------
def _softplus(x):
    return np.logaddexp(0.0, x).astype(np.float32)


def _causal_mask_tiles():
    kk = np.arange(KC)[:, None]
    qq = np.arange(QB)[None, :]
    tiles = np.stack(
        [(qq >= kk + m * KC) for m in range(QB // KC)]
    ).astype(ml_dtypes.bfloat16)
    return tiles  # [4, 128, 512]


def _is_causal(mask, neg=-2.3819763e38):
    m = mask.reshape(mask.shape[-2], mask.shape[-1])
    expect = np.where(
        np.tril(np.ones(m.shape, dtype=bool)), np.float32(0.0), np.float32(neg)
    )
    return np.array_equal(m, expect)


def prepare_inputs(hidden_states, mask, scaling, qkv_w, qkv_b, o_w, o_b):
    s = hidden_states.shape[1]
    hidden_states = np.asarray(hidden_states, dtype=np.float32)
    mask = np.asarray(mask, dtype=np.float32)
    scaling = np.asarray(scaling, dtype=np.float32)
    qkv_w = np.asarray(qkv_w, dtype=np.float32)
    qkv_b = np.asarray(qkv_b, dtype=np.float32)
    o_w = np.asarray(o_w, dtype=np.float32)
    o_b = np.asarray(o_b, dtype=np.float32)

    causal = bool(_is_causal(mask))
    bias = bool(np.any(qkv_b))

    scale = (1.442695041 / math.sqrt(HD)) * _softplus(scaling)  # [80]
    wq = qkv_w[:Q_SIZE] * np.tile(scale, NH)[:, None]           # scaled
    bq = qkv_b[:Q_SIZE] * np.tile(scale, NH)

    if causal:
        m01_full = _causal_mask_tiles()
    else:
        # exp(mask) transposed to [k, q], tiled as [s/128, 128, s]
        me = np.exp(mask.reshape(s, s).T.astype(np.float32))
        m01_full = np.ascontiguousarray(
            me.reshape(s // KC, KC, s)).astype(ml_dtypes.bfloat16)

    # xt depends only on batch; wt/owt only on head-group -> build each once
    xts = []
    hs_bf = hidden_states.astype(ml_dtypes.bfloat16)
    for b in range(B):
        xt = np.ascontiguousarray(hs_bf[b].T)                  # [1280, s] bf16
        if bias:
            xt = np.concatenate(
                [xt, np.ones((1, s), ml_dtypes.bfloat16)], axis=0)
        xts.append(xt)
    wtms, wtvps, wtbs, owts = [], [], [], []
    for hg in range(2):
        qrows = slice(hg * LQ, (hg + 1) * LQ)
        krows = slice(Q_SIZE + hg * LKV, Q_SIZE + (hg + 1) * LKV)
        vrows = slice(Q_SIZE + KV_SIZE + hg * LKV,
                      Q_SIZE + KV_SIZE + (hg + 1) * LKV)
        w_slice = np.concatenate(
            [wq[qrows], qkv_w[krows], qkv_w[vrows]], axis=0)   # [960, 1280]
        wtT = np.ascontiguousarray(w_slice.T)                  # [1280, 960]
        arr = wtT.reshape(HID // 128, 128, NQKV)               # [c, p, n]
        # m-major prepack: wtm[m, p, c, j] = wtT[c*128+p, m*128+j]
        wtm = np.zeros((7, 128, HID // 128, 128), dtype=np.float32)
        for m in range(7):
            mw = 128 if m < 6 else 32
            wtm[m, :, :, 0:mw] = arr[:, :, m * 128:m * 128 + mw].transpose(
                1, 0, 2)
        wtms.append(np.ascontiguousarray(wtm).astype(ml_dtypes.bfloat16))
        wtvps.append(np.ascontiguousarray(
            arr[:, :, LQ + LKV:NQKV].transpose(1, 0, 2)).astype(
            ml_dtypes.bfloat16))
        if bias:
            b_slice = np.concatenate([bq[qrows], qkv_b[krows], qkv_b[vrows]])
            wtbs.append(b_slice[None, :].astype(ml_dtypes.bfloat16))
        owts.append(np.ascontiguousarray(
            o_w[:, hg * LQ:(hg + 1) * LQ].T).astype(ml_dtypes.bfloat16))
    in_maps = []
    for c in range(NCORE):
        b, hg = divmod(c, 2)
        m = {"xt": xts[b], "wtm": wtms[hg], "wtvp": wtvps[hg],
             "owt": owts[hg], "m01": m01_full}
        if bias:
            m["wtb"] = wtbs[hg]
        in_maps.append(m)
    return in_maps, causal, bias, o_b


def kernel(hidden_states, mask, scaling, qkv_w, qkv_b, o_w, o_b):
    s = hidden_states.shape[1]
    in_maps, causal, bias, o_b32 = prepare_inputs(
        hidden_states, mask, scaling, qkv_w, qkv_b, o_w, o_b)
    run = _get_runner(s, causal, bias)
    res = run(in_maps)
    out = np.empty((B, s, HID), dtype=np.float32)
    for b in range(B):
        out[b] = (res[2 * b]["out"].astype(np.float32)
                  + res[2 * b + 1]["out"].astype(np.float32)
                  + o_b32[None, :])
    return out

